# revision 23
# baseline (speedup 1.0000x reference)
"""Causal self-attention (B=4, T=2048, C=1024, H=16) on 8 trn2 cores.

Sharding: batch (4-way) x head-group (2-way).  Core i handles batch i//2 and
heads [8*(i%2), 8*(i%2)+8).  Each core computes qkv projection for its head
slice, causal attention, and a partial out-projection (contraction over its
512 att columns).  Host sums the two partials per batch.

All matmul operands are bf16 (PE streams bf16 at ~1 col/cycle @2.4GHz; FWL
halves weight-load time).  PSUM accumulation stays fp32.

FUSED PIPELINE: the QKV projection is cut into 4 t-quarters.  Quarter 0 runs
standalone, then attention chunk j (tq span [512j, 512j+512)) is emitted with
quarter j+1's projection chains interleaved one-per-block into its head-pair
blocks.  The PE executes its queue in order, so the interleaved (always
ready) projection work fills the cycles where score matmuls wait on the
scalar engine's exp.  The scalar queue carries ONLY exps (plus early-quarter
psum->sbuf copies); all drain-dependent copies (softmax sums, y tiles) are
on the DVE so exps never head-of-line block.  Chunk j's out-projection tiles
are delayed and interleaved into chunk 3's blocks (chunks 0-2) or a short
tail (chunk 3).

PSUM (8 banks): fused region = proj chains 2 + st 4 + av 2.  Tail region
(after proj pool closes) = y 2 + st 4 + av 2.

Host-prepped inputs (bf16):
  - xT       (C, T)       : x[b].T
  - wqk      (8,128,8*128): per m-tile of [wq_g; wk_g].T, k-tiles along free
  - wv       (C, 512)     : wv_g.T
  - wo       (512, C)     : w_out.T row-slice for this head group
  - masks    (128, 1280)  : packed causal masks [512|384|256|128] variants
  - vinit    (128, 772)   : full v_sb tile init pattern (ones cols per
                            pair block; V regions overwritten by extracts)
  - ones2    (65, 128)    : K=65 lhsT broadcasting both heads' softmax sums
                            (row 0 -> out rows 0..63, row 64 -> 64..127)
Layouts on chip:
  - QT/KT  [128, 4, T]   rows = head-major (hl*64+d), T on free dim
  - V      [128, 16, 772]: per t-tile, per head pair [V_e|1] + [1|0*63|V_o]
  - attT   [128, 4, T]   rows = c_local = hl*64+d  (lhsT for out-proj)
"""

from contextlib import ExitStack

import numpy as np
import ml_dtypes

import concourse.bass as bass
import concourse.mybir as mybir
import concourse.tile as tile
from concourse import bacc, bass_utils

B, T, C, H, HD = 4, 2048, 1024, 16, 64
HG = 2  # head groups (tensor-parallel dim)
HPG = H // HG  # 8 heads per group
OG = HPG * HD  # 512: local width of q/k/v slice
KT_C = C // 128  # 8 contraction tiles for the projections
NT = T // 128  # 16 t-tiles
NQ = T // 512  # 4 tq chunks
PAIR_W = 65 + 128  # v_sb cols per head pair: [V_e|1] + [0*63|1|V_o]

f32 = mybir.dt.float32
bf16 = mybir.dt.bfloat16

TRACE = False  # test.py flips this for profiling runs
LAST_RUN = {}

_NC_CACHE = []


def _mm(nc, out, lhsT, rhs, **kw):
    nc.tensor.matmul(out, lhsT, rhs, **kw)


def _build_nc():
    nc = bacc.Bacc(trn_type="TRN2", target_bir_lowering=False, debug=False)
    xT = nc.dram_tensor("xT", [C, T], bf16, kind="ExternalInput").ap()
    wqk = nc.dram_tensor("wqk", [8, 128, 1024], bf16, kind="ExternalInput").ap()
    wv = nc.dram_tensor("wv", [C, OG], bf16, kind="ExternalInput").ap()
    wo = nc.dram_tensor("wo", [OG, C], bf16, kind="ExternalInput").ap()
    masks = nc.dram_tensor("masks", [128, 1280], bf16, kind="ExternalInput").ap()
    vinit = nc.dram_tensor("vinit", [128, 4 * PAIR_W], bf16, kind="ExternalInput").ap()
    ones2 = nc.dram_tensor("ones2", [65, 128], bf16, kind="ExternalInput").ap()
    y = nc.dram_tensor("y", [T, C], f32, kind="ExternalOutput").ap()

    with tile.TileContext(nc) as tc:
        _body(tc, nc, xT, wqk, wv, wo, masks, vinit, ones2, y)
    nc.compile()
    return nc


def _body(tc, nc, xT, wqk, wv, wo, masks, vinit, ones2, y):
    exp_f = mybir.ActivationFunctionType.Exp

    with (
        tc.tile_pool(name="persist", bufs=1) as persist,
        tc.tile_pool(name="wqk_p", bufs=1) as wqk_p,
        tc.tile_pool(name="wv_p", bufs=1) as wv_p,
        tc.tile_pool(name="wo_p", bufs=1) as wo_p,
        tc.tile_pool(name="mask_p", bufs=1) as mask_p,
        tc.tile_pool(name="ones_p", bufs=1) as ones_p,
        tc.tile_pool(name="sums_p", bufs=1) as sums_p,
        tc.tile_pool(name="pt_p", bufs=4) as pt_p,
        tc.tile_pool(name="bcast_p", bufs=3) as bcast_p,
        tc.tile_pool(name="xh_p", bufs=3) as xh_p,
    ):
        qt = persist.tile([128, 4, T], bf16)
        kt = persist.tile([128, 4, T], bf16)
        v_sb = persist.tile([128, NT, 4 * PAIR_W], bf16)
        attT = persist.tile([128, 4, T], bf16)

        # ---- initial DMAs: x quarter 0 first, then weights ----
        xh = {}

        def emit_loads(q):
            for k in range(KT_C):
                t = xh_p.tile([128, 512], bf16, tag=f"xh{k}", name=f"xh{q}_{k}")
                nc.sync.dma_start(
                    t[:], xT[k * 128 : (k + 1) * 128, q * 512 : (q + 1) * 512]
                )
                xh[(q, k)] = t

        # DMA priority: x quarter 0 + K-wave weights first (K chains run
        # first), then V weights, Q weights, then everything later-needed.
        wts = {}

        def load_wt(m):
            wt = wqk_p.tile([128, 1024], bf16, tag=f"wqk{m}", name=f"wt{m}")
            nc.sync.dma_start(wt[:], wqk[m, :, :])
            wts[m] = wt

        for k in range(KT_C):
            t = xh_p.tile([128, 512], bf16, tag=f"xh{k}", name=f"xh0_{k}")
            nc.sync.dma_start(t[:], xT[k * 128 : (k + 1) * 128, 0:512])
            xh[(0, k)] = t
            if k % 2 == 1:
                load_wt(4 + k // 2)
        wv_sb = wv_p.tile([128, KT_C, OG], bf16)
        for k in range(KT_C):
            nc.sync.dma_start(wv_sb[:, k, :], wv[k * 128 : (k + 1) * 128, :])
        for m in range(4):
            load_wt(m)
        # full-tile vinit writes: a strided sub-range DMA here races the
        # AV reads of the ones columns (rearranged DMA writes are not
        # reliably ordered against readers) -- write the whole tile and let
        # the V extracts overwrite their (tracked, overlapping) ranges.
        for tt in range(4):
            nc.sync.dma_start(v_sb[:, tt, :], vinit[:])
        mk = mask_p.tile([128, 1280], bf16)
        nc.sync.dma_start(mk[:], masks[:])
        ones2_sb = ones_p.tile([65, 128], bf16)
        nc.sync.dma_start(ones2_sb[:], ones2[:])
        wo_sb = wo_p.tile([128, 4, C], bf16)
        for k in range(4):
            nc.sync.dma_start(wo_sb[:, k, :], wo[k * 128 : (k + 1) * 128, :])
        for tt in range(4, NT):
            nc.sync.dma_start(v_sb[:, tt, :], vinit[:])
        sums_tiles = [
            sums_p.tile([65, 512], bf16, name=f"sums{i}") for i in (0, 1, 2)
        ]
        for st_ in sums_tiles:
            nc.vector.memset(st_[:], 0.0)
        # warm the ACT exp table set (~2.7us) during the DMA-bound ramp
        # instead of at chunk 0's first real exp; scratch has no readers
        scr = sums_p.tile([1, 8], bf16, name="expwarm")
        nc.scalar.activation(
            scr[:], sums_tiles[0][0:1, 0:8], exp_f, scale=0.125
        )

        def head_ctx(hl):
            """Slices/layout facts for local head hl."""
            p0 = (hl % 2) * 64
            mt = hl // 2
            qrow = slice(p0, p0 + 64)
            vb0 = (hl // 2) * PAIR_W
            if hl % 2 == 0:
                vsl = (vb0, vb0 + 65)  # [V|1] -> rows 0..64
                srow, arow = 64, slice(0, 64)
            else:
                vsl = (vb0 + 65, vb0 + 193)  # [1|0*63|V] -> row 0 sums, 64..127 att
                srow, arow = 0, slice(64, 128)
            return p0, mt, qrow, vsl, srow, arow

        with (
            tc.tile_pool(name="st_ps", bufs=2, space="PSUM") as st_ps,
            tc.tile_pool(name="av_ps", bufs=2, space="PSUM") as av_ps,
        ):

            def do_chunk(j, fillers, alloc_bps):
                """Attention for tq chunk j; fillers are emitted spread over
                the head-pair blocks (ready PE work that absorbs stalls)."""
                ntk = 4 * j + 4
                ng = ntk // 2
                tq = slice(j * 512, (j + 1) * 512)
                nb = HPG // 2
                for bi, ha in enumerate(range(0, HPG, 2)):
                    for f in fillers[
                        bi * len(fillers) // nb : (bi + 1) * len(fillers) // nb
                    ]:
                        f()
                    ctxs = [head_ctx(ha), head_ctx(ha + 1)]
                    pts = {0: [None] * ng, 1: [None] * ng}

                    def emit_pair(s, g):
                        _, mt, qrow, _, _, _ = ctxs[s]
                        # diagonal tiles only need tq >= tk: narrow the
                        # st/exp/av width (512/384/256/128) instead of
                        # masking fully-computed tiles.
                        geom = []  # per u: (tq_off, width, pt_col)
                        pcol = 0
                        for u in range(2):
                            tk = 2 * g + u
                            v = tk - 4 * j
                            off = 128 * v if v > 0 else 0
                            w = 512 - off
                            if u == 1 and pcol == 512:
                                pcol = 512
                            geom.append((off, w, pcol))
                            pcol = 512 if u == 0 and w == 512 else pcol + w
                        dg = 2 * g - 4 * j
                        ps = st_ps.tile([128, 1024], f32, tag="st")
                        for u in range(2):
                            off, w, pc = geom[u]
                            tk = 2 * g + u
                            _mm(
                                nc,
                                ps[:, pc : pc + w],
                                kt[qrow, mt, tk * 128 : (tk + 1) * 128],
                                qt[qrow, mt, j * 512 + off : (j + 1) * 512],
                                start=True,
                                stop=True,
                            )
                        tot = geom[1][2] + geom[1][1]
                        pt = pt_p.tile([128, 1024], bf16, tag=f"pt{s}")
                        nc.scalar.activation(
                            pt[:, 0:tot], ps[:, 0:tot], exp_f, scale=0.125
                        )
                        if dg == 0:  # pair (4j, 4j+1): widths 512|384
                            nc.vector.tensor_mul(
                                pt[:, 0:896], pt[:, 0:896], mk[:, 0:896]
                            )
                        elif dg == 2:  # pair (4j+2, 4j+3): widths 256|128
                            nc.vector.tensor_mul(
                                pt[:, 0:384], pt[:, 0:384], mk[:, 896:1280]
                            )
                        pts[s][g] = (pt, geom)

                    avs = [
                        av_ps.tile([128, 512], f32, tag="av", name=f"av{s}_{ha}_{j}")
                        for s in (0, 1)
                    ]
                    emit_pair(0, 0)
                    emit_pair(1, 0)
                    for g in range(ng):
                        if g + 1 < ng:
                            # alternate head order so the 2nd head's st psum
                            # (freed by that head's g-1 exp, which ran FIRST
                            # in the previous unit) is ready at dispatch --
                            # lets its K=64 pair pack into the other row group
                            for s_ in ((0, 1) if g % 2 else (1, 0)):
                                emit_pair(s_, g + 1)
                        for u in range(2):
                            for s in (0, 1):
                                _, _, _, vsl, _, _ = ctxs[s]
                                pt, geom = pts[s][g]
                                off, w, pc = geom[u]
                                tk = 2 * g + u
                                _mm(
                                    nc,
                                    avs[s][0 : vsl[1] - vsl[0], off : off + w],
                                    v_sb[:, tk, vsl[0] : vsl[1]],
                                    pt[:, pc : pc + w],
                                    start=(tk == 0),
                                    stop=(tk == ntk - 1),
                                )

                    # softmax denominators: both heads' sums rows -> rows
                    # 0/64 of a [65,512] tile (zeros between), one K=65
                    # broadcast matmul fans them out to out rows 0-63 (even)
                    # / 64-127 (odd), fast-reciprocal, apply on DVE.
                    sums2 = sums_tiles[(ha // 2) % 3]
                    nc.vector.tensor_copy(sums2[0:1, :], avs[0][64:65, :])
                    nc.vector.tensor_copy(sums2[64:65, :], avs[1][0:1, :])
                    bps = alloc_bps(ha, j)
                    _mm(
                        nc,
                        bps,
                        ones2_sb[:],
                        sums2[:],
                        start=True,
                        stop=True,
                    )
                    bc = bcast_p.tile([128, 512], f32, tag="bc")
                    nc.vector.reciprocal_approx_fast(bc[:], bps)
                    for s in (0, 1):
                        _, mt, _, _, _, arow = ctxs[s]
                        nc.vector.tensor_mul(
                            attT[arow, mt, tq], avs[s][arow, :], bc[arow, :]
                        )

            # ---- fused region: projections (2 psum banks) + attention ----
            with tc.tile_pool(name="p1ps", bufs=2, space="PSUM") as p1ps:

                def emit_proj_chain(kind, q, idx, on_scalar):
                    ps = p1ps.tile(
                        [128, 512], f32, tag="p1", name=f"p1_{kind}{q}_{idx}"
                    )
                    t0 = q * 512
                    if kind in ("q", "k"):
                        wt = wts[(0 if kind == "q" else 4) + idx]
                        for k in range(KT_C):
                            _mm(
                                nc,
                                ps[:],
                                wt[:, k * 128 : (k + 1) * 128],
                                xh[(q, k)][:],
                                start=(k == 0),
                                stop=(k == KT_C - 1),
                            )
                        dst = (qt if kind == "q" else kt)[:, idx, t0 : t0 + 512]
                        if on_scalar:
                            nc.scalar.copy(dst, ps[:])
                        else:
                            nc.vector.tensor_copy(dst, ps[:])
                    else:  # v: out rows t, free = o (head-major)
                        tt = 4 * q + idx
                        for k in range(KT_C):
                            _mm(
                                nc,
                                ps[:],
                                xh[(q, k)][:, idx * 128 : (idx + 1) * 128],
                                wv_sb[:, k, :],
                                start=(k == 0),
                                stop=(k == KT_C - 1),
                            )
                        src_e = ps[:].rearrange("p (h d) -> p h d", d=64)[:, 0::2, :]
                        src_o = ps[:].rearrange("p (h d) -> p h d", d=64)[:, 1::2, :]
                        dstv = v_sb[:, tt, :].rearrange("p (q w) -> p q w", w=PAIR_W)
                        nc.vector.tensor_copy(dstv[:, :, 0:64], src_e)
                        nc.vector.tensor_copy(dstv[:, :, 129:193], src_o)

                def proj_fillers(q, on_scalar):
                    fs = []
                    for kind in ("k", "v", "q"):
                        for idx in range(4):
                            fs.append(
                                lambda kind=kind, idx=idx: emit_proj_chain(
                                    kind, q, idx, on_scalar
                                )
                            )
                    return fs

                def bps_p1(ha, j):
                    t = p1ps.tile([128, 512], f32, tag="p1", name=f"bps_{ha}_{j}")
                    return t[:]

                # quarter 0 standalone (copies on scalar: it is idle here)
                for f in proj_fillers(0, True):
                    f()
                emit_loads(1)
                emit_loads(2)
                do_chunk(0, proj_fillers(1, True), bps_p1)
                emit_loads(3)
                do_chunk(1, proj_fillers(2, False), bps_p1)
                do_chunk(2, proj_fillers(3, False), bps_p1)

            # ---- tail region: chunk 3 + delayed out-projections ----
            with (
                tc.tile_pool(name="y_ps", bufs=1, space="PSUM") as y_ps,
                tc.tile_pool(name="yo_p", bufs=4) as yo_p,
            ):

                def do_outproj_tile(tt):
                    yps = y_ps.tile([128, 1024], f32, tag="y", name=f"yps_{tt}")
                    pso = [yps[:, 0:512], yps[:, 512:1024]]
                    for k in range(4):
                        for o in range(2):
                            _mm(
                                nc,
                                pso[o],
                                attT[:, k, tt * 128 : (tt + 1) * 128],
                                wo_sb[:, k, o * 512 : (o + 1) * 512],
                                start=(k == 0),
                                stop=(k == 3),
                            )
                    for o in range(2):
                        yo = yo_p.tile([128, 512], f32, tag="yo", name=f"yo_{tt}_{o}")
                        nc.vector.tensor_copy(yo[:], pso[o])
                        nc.sync.dma_start(
                            y[tt * 128 : (tt + 1) * 128, o * 512 : (o + 1) * 512],
                            yo[:],
                        )

                def bps_y(ha, j):
                    t = y_ps.tile([128, 1024], f32, tag="y", name=f"bps_{ha}_{j}")
                    return t[:, 0:512]

                op_fillers = [
                    (lambda tt=tt: do_outproj_tile(tt)) for tt in range(12)
                ]
                do_chunk(3, op_fillers, bps_y)
                for tt in range(12, 16):
                    do_outproj_tile(tt)


def _host_prep(x, w_qkv, w_out):
    xT_all = np.ascontiguousarray(x.transpose(0, 2, 1)).astype(ml_dtypes.bfloat16)
    # packed diagonal masks, all variant-0 (keep iff tq_local >= tk_local):
    # [0:512) pair1-u0 w=512, [512:896) pair1-u1 w=384,
    # [896:1152) pair2-u0 w=256, [1152:1280) pair2-u1 w=128
    tk_l = np.arange(128)[:, None]
    m0 = (np.arange(512)[None, :] >= tk_l).astype(np.float32)
    masks = np.concatenate([m0, m0[:, :384], m0[:, :256], m0[:, :128]], axis=1).astype(
        ml_dtypes.bfloat16
    )

    per_group = []
    for g in range(HG):
        wq = w_qkv[g * OG : (g + 1) * OG]
        wk = w_qkv[C + g * OG : C + (g + 1) * OG]
        wvg = w_qkv[2 * C + g * OG : 2 * C + (g + 1) * OG]
        wqkT = np.concatenate([wq, wk], axis=0).T  # (C, 1024)
        # wqk_r[m, p, k*128+j] = wqkT[k*128+p, m*128+j]
        wqk_r = np.ascontiguousarray(
            wqkT.reshape(8, 128, 8, 128).transpose(2, 1, 0, 3).reshape(8, 128, 1024)
        ).astype(ml_dtypes.bfloat16)
        wv_t = np.ascontiguousarray(wvg.T).astype(ml_dtypes.bfloat16)  # (C, 512)
        wo_t = np.ascontiguousarray(w_out.T[g * OG : (g + 1) * OG]).astype(
            ml_dtypes.bfloat16
        )  # (512, C)
        per_group.append((wqk_r, wv_t, wo_t))
    vinit = np.zeros((128, 4 * PAIR_W), np.float32)
    for pr in range(4):
        vinit[:, pr * PAIR_W + 64] = 1.0  # even-head ones col
        vinit[:, pr * PAIR_W + 65] = 1.0  # odd-head ones col (block col 0)
    vinit = vinit.astype(ml_dtypes.bfloat16)
    ones2 = np.zeros((65, 128), np.float32)
    ones2[0, 0:64] = 1.0  # even head sums (row 0) -> out rows 0..63
    ones2[64, 64:128] = 1.0  # odd head sums (row 64) -> out rows 64..127
    ones2 = ones2.astype(ml_dtypes.bfloat16)
    return xT_all, masks, vinit, ones2, per_group


def kernel(x, w_qkv, w_out):
    x = np.asarray(x)
    w_qkv = np.asarray(w_qkv)
    w_out = np.asarray(w_out)
    xT_all, masks, vinit, ones2, per_group = _host_prep(x, w_qkv, w_out)

    if not _NC_CACHE:
        _NC_CACHE.append(_build_nc())
    nc = _NC_CACHE[0]

    in_maps = []
    for core in range(8):
        b, g = core // 2, core % 2
        wqk_r, wv_t, wo_t = per_group[g]
        in_maps.append(
            {"xT": xT_all[b], "wqk": wqk_r, "wv": wv_t, "wo": wo_t, "masks": masks,
             "vinit": vinit, "ones2": ones2}
        )

    try:
        res = bass_utils.run_bass_kernel_spmd(
            nc, in_maps, core_ids=list(range(8)), trace=TRACE
        )
    except Exception:
        # one retry: a previously aborted process can leave a core wedged
        # (NRT_EXEC_UNIT_UNRECOVERABLE) on the first execute
        res = bass_utils.run_bass_kernel_spmd(
            nc, in_maps, core_ids=list(range(8)), trace=TRACE
        )
    LAST_RUN["res"] = res

    y = np.empty((B, T, C), np.float32)
    for b in range(B):
        y[b] = res.results[2 * b]["y"] + res.results[2 * b + 1]["y"]
    return y
